# revision 3
# baseline (speedup 1.0000x reference)
"""KMeans min-distance loss kernel for Trainium2 (8 NeuronCores, SPMD).

Problem: features [262144, 128] f32, centers [256, 128] f32.
  d2[n,k] = ||f_n||^2 + ||c_k||^2 - 2 f_n.c_k ; out = mean_n sqrt(min_k d2)

Sharding: data-parallel over N (32768 rows per core), centers replicated.
Each core returns [128] partial sums of min-distances; host reduces.

Per-core pipeline:
  - SWDGE cast-DMA 1MB groups: f32 dram -> bf16 sbuf [128p, 16, 128]
  - PE transpose (bf16) chunks -> pt in PSUM, ACT copy-casts to fp8
    DoubleRow stationary [128, 2, TG, 128] whose plane 1 carries two
    static rows of ones (partitions 0-1)
  - single fp8e4 DoubleRow matmul per chunk, contraction 256 =
    128 feature dims + 2 rows carrying an e4m3 decomposition of
    ||c_k||^2  ->  PSUM [128n, 256k] f32 holds c2 - 2 f.c directly
    (no PSUM preload matmuls, no accumulation)
  - min over k: alternate per 4-chunk batch between
      (a) DVE tensor_reduce(min) straight from PSUM f32
      (b) ACT f32->bf16 batched copy + DVE bf16 TT-min cascade at 2x
    both write bf16 min values
  - f2 = ||f||^2: one batched DVE square (TT mult) + one segmented
    reduce-add per 16-chunk group
  - tail: sqrt(m + f2) with ACT accumulation -> [128] sums -> DMA out
"""

import sys

for p in ("/opt/trn_rl_repo", "/opt/trn_rl_repo/concourse"):
    if p not in sys.path:
        sys.path.insert(0, p)

import numpy as np

N_TOTAL = 262144
K = 256
D = 128
N_CORES = 8
N_PER_CORE = N_TOTAL // N_CORES  # 32768
P = 128
CHUNKS = N_PER_CORE // P         # 256 chunks of 128 rows
G = 32                           # chunks per DMA group (2 MB f32 read)
GROUPS = CHUNKS // G             # 8
TG = 4                           # chunks per transpose/psum batch
NFT = 4                          # persistent fT stationary buffers
ACT_MOD = 2                      # every ACT_MOD-th batch drains via ACT

_compiled = None


def _build(repeat: int = 1):
    import concourse.bass as bass
    import concourse.bacc as bacc
    import concourse.tile as tile
    from concourse import mybir

    f32 = mybir.dt.float32
    bf16 = mybir.dt.bfloat16
    fp8 = mybir.dt.float8e4
    Alu = mybir.AluOpType
    Act = mybir.ActivationFunctionType
    PM = mybir.MatmulPerfMode

    nc = bacc.Bacc(
        "TRN2", target_bir_lowering=False, debug=False, num_devices=N_CORES
    )

    feats = nc.dram_tensor("features", [N_PER_CORE, D], f32, kind="ExternalInput").ap()
    ctdr = nc.dram_tensor("ctdr", [D, 2, K], fp8, kind="ExternalInput").ap()
    ident = nc.dram_tensor("ident", [P, P], bf16, kind="ExternalInput").ap()
    out = nc.dram_tensor("out", [P, 1], f32, kind="ExternalOutput").ap()

    with tile.TileContext(nc) as tc:
        with (
            tc.tile_pool(name="consts", bufs=1) as consts,
            tc.tile_pool(name="featg", bufs=3) as featg_pool,
            tc.tile_pool(name="dist2", bufs=3) as dist2_pool,
            tc.tile_pool(name="dumps", bufs=2) as dumps,
            tc.tile_pool(name="coll", bufs=1) as coll,
            tc.tile_pool(name="ptrans", bufs=2, space="PSUM") as ptrans_pool,
            tc.tile_pool(name="pcross", bufs=3, space="PSUM") as pcross_pool,
        ):
            ct_s = consts.tile([D, 2, K], fp8)
            nc.sync.dma_start(ct_s[:], ctdr)
            id_s = consts.tile([P, P], bf16)
            nc.sync.dma_start(id_s[:], ident)

            m_coll = coll.tile([P, CHUNKS], bf16)
            f2_coll = coll.tile([P, CHUNKS], f32)

            # Persistent DoubleRow stationary buffers [K, 2, TG, M].
            # Plane 1 is static: rows 0..1 = 1.0 (the c2 rows), rest 0.
            ft_bufs = []
            for b in range(NFT):
                ft = consts.tile([P, 2, TG, P], fp8, tag=f"ftdr{b}")
                nc.vector.memset(ft[:, 1, :, :], 0.0)
                nc.vector.memset(ft[0:2, 1, :, :], 1.0)
                ft_bufs.append(ft)

            fview = feats.rearrange("(g p c) d -> g p c d", p=P, c=G)

            for g in range(GROUPS * repeat):
                g = g % GROUPS
                fg = featg_pool.tile([P, G, D], bf16)
                nc.gpsimd.dma_start(fg[:], fview[g])  # SWDGE cast f32->bf16

                # f2 per group: batched square + segmented sum (2 DVE insts)
                sq = dumps.tile([P, G, D], bf16, tag="sq")
                nc.vector.tensor_tensor(
                    out=sq[:], in0=fg[:], in1=fg[:], op=Alu.mult,
                )
                nc.vector.tensor_reduce(
                    out=f2_coll[:, g * G : (g + 1) * G], in_=sq[:],
                    axis=mybir.AxisListType.X, op=Alu.add,
                )

                for cb in range(G // TG):
                    tg_idx = g * (G // TG) + cb
                    ft = ft_bufs[tg_idx % NFT]
                    ib = g * G + cb * TG

                    pt = ptrans_pool.tile([D, TG, P], bf16)
                    for j in range(TG):
                        nc.tensor.transpose(
                            pt[:, j, :], fg[:, cb * TG + j, :], id_s[:]
                        )
                    # ACT: cast bf16 -> fp8 into plane 0 of the stationary
                    nc.scalar.copy(ft[:, 0, :, :], pt[:])

                    px4 = pcross_pool.tile([P, TG, K], f32)
                    for j in range(TG):
                        nc.tensor.matmul(
                            px4[:, j, :], ft[:, :, j, :], ct_s[:],
                            start=True, stop=True,
                            perf_mode=PM.DoubleRow,
                            skip_group_check=True,
                        )

                    if ACT_MOD and (tg_idx % ACT_MOD == 0):
                        # ACT drains PSUM -> bf16 SBUF; DVE min-cascade at 2x
                        d2t = dist2_pool.tile([P, TG, K], bf16)
                        nc.scalar.copy(d2t[:], px4[:])
                        h1 = dumps.tile([P, TG, K // 2], bf16, tag="h1")
                        nc.vector.tensor_tensor(
                            out=h1[:], in0=d2t[:, :, : K // 2],
                            in1=d2t[:, :, K // 2 :], op=Alu.min,
                        )
                        h2 = dumps.tile([P, TG, K // 4], bf16, tag="h2")
                        nc.vector.tensor_tensor(
                            out=h2[:], in0=h1[:, :, : K // 4],
                            in1=h1[:, :, K // 4 :], op=Alu.min,
                        )
                        nc.vector.tensor_reduce(
                            out=m_coll[:, ib : ib + TG], in_=h2[:],
                            axis=mybir.AxisListType.X, op=Alu.min,
                        )
                    else:
                        nc.vector.tensor_reduce(
                            out=m_coll[:, ib : ib + TG], in_=px4[:],
                            axis=mybir.AxisListType.X, op=Alu.min,
                        )

            # tail: sums[p] = sum_i sqrt(m[p,i] + f2[p,i])
            d2t = coll.tile([P, CHUNKS], f32)
            nc.vector.tensor_add(d2t[:], m_coll[:], f2_coll[:])
            dist = coll.tile([P, CHUNKS], f32)
            sums = coll.tile([P, 1], f32)
            nc.scalar.activation(dist[:], d2t[:], Act.Sqrt, accum_out=sums[:])
            nc.sync.dma_start(out, sums[:])

    nc.compile()
    return nc


def _get_compiled():
    global _compiled
    if _compiled is None:
        _compiled = _build()
    return _compiled


def _make_aux(centers: np.ndarray):
    import ml_dtypes

    e4 = ml_dtypes.float8_e4m3fn
    # plane 0: a[d,k] = e4m3(-2*c[k,d]); effective center = -a/2
    a = (-2.0 * centers.T).astype(e4)                              # [D, K]
    c_eff = -(a.astype(np.float64)) / 2.0
    c2_eff = (c_eff ** 2).sum(axis=0)                              # [K]
    r0 = c2_eff.astype(e4)
    r1 = (c2_eff - r0.astype(np.float64)).astype(e4)
    ctdr = np.zeros((D, 2, K), dtype=e4)
    ctdr[:, 0, :] = a
    ctdr[0, 1, :] = r0
    ctdr[1, 1, :] = r1
    ident = np.eye(P, dtype=ml_dtypes.bfloat16)
    return ctdr, ident


def _make_in_maps(features: np.ndarray, centers: np.ndarray):
    ctdr, ident = _make_aux(centers)
    return [
        {
            "features": features[c * N_PER_CORE : (c + 1) * N_PER_CORE],
            "ctdr": ctdr,
            "ident": ident,
        }
        for c in range(N_CORES)
    ]


def kernel(features: np.ndarray, centers: np.ndarray) -> np.ndarray:
    features = np.ascontiguousarray(np.asarray(features, dtype=np.float32))
    centers = np.ascontiguousarray(np.asarray(centers, dtype=np.float32))
    assert features.shape == (N_TOTAL, D) and centers.shape == (K, D)

    from concourse.bass_utils import run_bass_kernel_spmd

    nc = _get_compiled()
    in_maps = _make_in_maps(features, centers)
    res = run_bass_kernel_spmd(nc, in_maps, list(range(N_CORES)))
    total = 0.0
    for r in res.results:
        total += np.sum(r["out"].astype(np.float64))
    return np.float32(total / N_TOTAL)


if __name__ == "__main__":
    rng = np.random.default_rng(0)
    f = rng.standard_normal((N_TOTAL, D), dtype=np.float32)
    c = rng.standard_normal((K, D), dtype=np.float32)
    print(kernel(f, c))
